# revision 17
# baseline (speedup 1.0000x reference)
"""TreeLSTM (nn_BinaryTreeLSTM, S=128 B=512 D=256) Trainium2 kernel.

8-core data-parallel over the batch: each NeuronCore owns 64 batch rows and
runs the full sequential 128-step scan locally (no cross-core comms), per the
sharding hint. Host side converts the one-hot child masks to indices, lays
tensors out feature-on-partition, runs one SPMD NEFF on cores 0-7 via
bass_utils.run_bass_kernel_spmd, and reassembles the full [512, 128, 256]
fp32 output.

Math (identical to the reference):
  xp[s] = x[s] @ Wx[g].T + bx                    (4 gates, state-independent)
  per step s:  lh/lc, rh/rc = (h/c)[b, li[s,b]], (h/c)[b, ri[s,b]]
    gates = concat(lh, rh) @ W2 + xp[s]          (W2 = [Wlh; Wrh], one matmul)
    u = tanh(.); i, lf, rf, o = sigmoid(.)       (lf/rf share the xp f-term)
    cn = i*u + lf*lc + rf*rc; hn = o*tanh(cn)
    h[b, s] = m*hn; c[b, s] = m*cn               (rows start at zero)

Device design (per core):
  - State lives in SBUF, feature-on-partition: h fp16 [128, 8192, 2] and
    c fp32 [128, 8192, 2], elem = s*64+b, last dim = d//128.
  - Child rows are fetched per step with one gpsimd.ap_gather per state
    (128 indices = 64 left + 64 right, same list replicated per Q7 core).
  - Recurrent gate matmuls: weight-stationary fp16 [128,128] chunks x
    gathered rhs [128, 64], 10 gate-chunks x 4 K-chunks accumulated in PSUM.
  - Gate math on ACT (tanh/sigmoid over chunk-contiguous slices) + DVE;
    mask-multiply is fused into the state writebacks (h cast to fp16).
  - xp is precomputed on-device into 16 DRAM step-blocks and streamed back
    per step. It uses 3-term fp16 compensation (xh@Wh + xl@Wh + xh@Wl) for
    fp32-accurate xp, and the blocks are emitted interleaved with the scan
    so the extra PE work hides in scan idle slots.
  - Output h rows stream to DRAM as fp16 per step; host restores fp32/layout.

Precision: fp32 everywhere except fp16 matmul operands/state-h and the fp16
output rows. Measured vs the fp32 reference: fro-rel ~3.5e-4, absmax-rel
~1.5e-2 (fp16 noise random-walking through the 128-step recurrence).
"""

import numpy as np

import concourse.bass as bass
import concourse.mybir as mybir
import concourse.tile as tile
from concourse import bacc
from concourse import bass_utils

S, B, D = 128, 512, 256
NCORES = 8
BS = B // NCORES          # 64 batch rows per core
NE = S * BS               # 8192 state rows per core
GD = 5 * D                # 1280 recurrent gate outputs (u,i,lf,rf,o)
XD = 4 * D                # 1024 xp outputs (cx,ix,fx,ox)
NMC = GD // 128           # 10 gate chunks
XMC = XD // 128           # 8 xp chunks
# psum gate chunk -> xp chunk (rf reuses the f projection)
XP_MAP10 = [0, 1, 2, 3, 4, 5, 4, 5, 6, 7]

BF16 = mybir.dt.float16  # "BF16" name kept; fp16 chosen for 11-bit mantissa at same PE rate
F32 = mybir.dt.float32
I16 = mybir.dt.int16
AF = mybir.ActivationFunctionType
OP = mybir.AluOpType

_CACHED = {}


def build_program():
    """Trace + compile the per-core Bass program (same NEFF on all 8 cores)."""
    nc = bacc.Bacc("TRN2", target_bir_lowering=False, debug=False)

    d_xTh = nc.dram_tensor("xTh", [128, 2, NE], BF16, kind="ExternalInput").ap()
    d_xTl = nc.dram_tensor("xTl", [128, 2, NE], BF16, kind="ExternalInput").ap()
    d_w2 = nc.dram_tensor("w2", [128, 4 * NMC, 128], BF16, kind="ExternalInput").ap()
    d_wxh = nc.dram_tensor("wxh", [128, 2 * XMC, 128], BF16, kind="ExternalInput").ap()
    d_wxl = nc.dram_tensor("wxl", [128, 2 * XMC, 128], BF16, kind="ExternalInput").ap()
    d_bx = nc.dram_tensor("bx8", [128, XMC], F32, kind="ExternalInput").ap()
    d_idx = nc.dram_tensor("idx", [128, S, 8], I16, kind="ExternalInput").ap()
    d_mask = nc.dram_tensor("maskv", [1, NE], BF16, kind="ExternalInput").ap()
    d_out = nc.dram_tensor("hT", [S, 2, 128, BS], BF16, kind="ExternalOutput").ap()

    with tile.TileContext(nc) as tc:
        with tc.tile_pool(name="dram", bufs=1, space="DRAM") as dpool:
            xp_blocks = [dpool.tile([8, NMC, 128, BS], F32, name=f"xpb{i}", tag=f"xpb{i}")
                         for i in range(16)]

            # ---------------- phase A: xp = x @ Wx + bx ----------------
            # 3-term fp16 compensation: xp ~= xh@Wh + xl@Wh + xh@Wl (fp32-exact
            # to ~1e-6). Emitted in 16 step-blocks, interleaved into the scan
            # so the PE work hides in scan idle slots.
            phA_cm = tc.tile_pool(name="phA", bufs=1)
            phA = phA_cm.__enter__()
            psA_cm = tc.tile_pool(name="psA", bufs=2, space="PSUM")
            psA = psA_cm.__enter__()
            stA_cm = tc.tile_pool(name="stA", bufs=4)
            stA = stA_cm.__enter__()
            xbA_cm = tc.tile_pool(name="xbA", bufs=2)
            xbA = xbA_cm.__enter__()
            s_wxh = phA.tile([128, 2 * XMC, 128], BF16)
            s_wxl = phA.tile([128, 2 * XMC, 128], BF16)
            s_bx = phA.tile([128, XMC], F32)
            nc.sync.dma_start(out=s_wxh[:], in_=d_wxh[:])
            nc.sync.dma_start(out=s_wxl[:], in_=d_wxl[:])
            nc.sync.dma_start(out=s_bx[:], in_=d_bx[:])

            NCH = 16            # 16 column chunks of 512 (s,b) elements
            CW = NE // NCH      # 512

            def emit_xp_block(nch):
                xh = xbA.tile([128, 2, CW], BF16, name=f"xh{nch}", tag="xh")
                xl = xbA.tile([128, 2, CW], BF16, name=f"xl{nch}", tag="xl")
                nc.sync.dma_start(out=xh[:], in_=d_xTh[:, :, nch * CW:(nch + 1) * CW])
                nc.sync.dma_start(out=xl[:], in_=d_xTl[:, :, nch * CW:(nch + 1) * CW])
                for mc in range(XMC):
                    pst = psA.tile([128, CW], F32, name=f"pstA{nch}_{mc}", tag="pstA")
                    first = True
                    for kc in range(2):
                        for wmat, xmat in ((s_wxh, xh), (s_wxl, xh), (s_wxh, xl)):
                            nc.tensor.matmul(
                                pst[:],
                                lhsT=wmat[:, mc * 2 + kc, :],
                                rhs=xmat[:, kc, :],
                                start=first,
                                stop=(kc == 1 and xmat is xl),
                            )
                            first = False
                    stg = stA.tile([128, CW], F32, name=f"stg{nch}_{mc}", tag="stg")
                    if mc % 2 == 0:
                        nc.vector.tensor_scalar_add(stg[:], pst[:], s_bx[:, mc:mc + 1])
                    else:
                        nc.scalar.activation(stg[:], pst[:], AF.Identity,
                                             bias=s_bx[:, mc:mc + 1])
                    for pos in [i for i, x in enumerate(XP_MAP10) if x == mc]:
                        dst = xp_blocks[nch][:, pos, :, :]
                        nc.sync.dma_start(
                            out=dst.rearrange("s p b -> p s b"),
                            in_=stg[:].rearrange("p (s b) -> p s b", s=8),
                        )

            emit_xp_block(0)
            emit_xp_block(1)

            # --- persistent SBUF (allocated after phase A frees its pool) ---
            import contextlib
            _pstack = contextlib.ExitStack()
            persist = _pstack.enter_context(tc.tile_pool(name="persist", bufs=1))
            st_h = persist.tile([128, NE, 2], BF16)    # h state (fp16)
            st_c = persist.tile([128, NE, 2], F32)     # c state (fp32)
            s_w2 = persist.tile([128, 4 * NMC, 128], BF16)
            s_idx = persist.tile([128, S, 8], I16)
            s_mask = persist.tile([128, NE], F32)

            nc.vector.memset(st_h[:], 0.0)
            nc.vector.memset(st_c[:], 0.0)
            nc.sync.dma_start(out=s_w2[:], in_=d_w2[:])
            nc.sync.dma_start(out=s_idx[:], in_=d_idx[:])
            mask_bcast = bass.AP(
                tensor=d_mask.tensor,
                offset=d_mask.offset,
                ap=[[0, 128]] + list(d_mask.ap[1:]),
            )
            nc.gpsimd.dma_start(out=s_mask[:], in_=mask_bcast)

            # ---------------- phase B: the scan ----------------
            # Per-gate PSUM tiles + per-gate adds/activations: each gate's
            # epilogue starts as soon as its own 8 matmuls stop, so the cn/hn
            # chain pipelines INSIDE the matmul stream instead of after it.
            with (
                tc.tile_pool(name="gpool", bufs=2) as gpool,
                tc.tile_pool(name="xpool", bufs=6) as xpool,
                tc.tile_pool(name="gate", bufs=2) as gate,
                tc.tile_pool(name="psB", bufs=1, space="PSUM") as psB,
            ):
                for s in range(S):
                    if s % 8 == 0 and s // 8 + 2 < NCH:
                        emit_xp_block(s // 8 + 2)
                    xpt = xpool.tile([128, NMC * BS], F32, tag="xpt")
                    nc.sync.dma_start(
                        out=xpt[:].rearrange("p (t b) -> p t b", t=NMC),
                        in_=xp_blocks[s // 8][s % 8].rearrange("t p b -> p t b"),
                    )

                    gh = gpool.tile([128, 2 * BS, 2], BF16, tag="gh")
                    nc.gpsimd.ap_gather(
                        gh[:], st_h[:], s_idx[:, s, :],
                        channels=128, num_elems=NE, d=2, num_idxs=2 * BS,
                    )
                    gc = gpool.tile([128, 2 * BS, 2], F32, tag="gc")
                    nc.gpsimd.ap_gather(
                        gc[:], st_c[:], s_idx[:, s, :],
                        channels=128, num_elems=NE, d=2, num_idxs=2 * BS,
                    )

                    mrow = s_mask[:, s * BS:(s + 1) * BS]
                    mb = bass.AP(
                        tensor=mrow.tensor,
                        offset=mrow.offset,
                        ap=[mrow.ap[0], [0, 2]] + list(mrow.ap[1:]),
                    )
                    rows = slice(s * BS, (s + 1) * BS)
                    lc = gc[:, 0:BS, :].rearrange("p i c -> p c i")
                    rc = gc[:, BS:2 * BS, :].rearrange("p i c -> p c i")

                    tg = {}       # gate activations, [128, 2*BS] each
                    cn = gate.tile([128, 2 * BS], F32, tag="cn")
                    t2 = gate.tile([128, 2 * BS], F32, tag="t2")
                    t3 = gate.tile([128, 2 * BS], F32, tag="t3")
                    tc_t = gate.tile([128, 2 * BS], F32, tag="tc_t")
                    for g in range(5):      # u, i, lf, rf, o
                        psg = psB.tile([128, 2 * BS], F32, name=f"psg{g}_{s}",
                                       tag=f"psg{g}")
                        for mc2 in range(2):
                            mc = g * 2 + mc2
                            for kc in range(4):  # (lr, dhi)
                                lr, dhi = divmod(kc, 2)
                                nc.tensor.matmul(
                                    psg[:, mc2 * BS:(mc2 + 1) * BS],
                                    lhsT=s_w2[:, mc * 4 + kc, :],
                                    rhs=gh[:, lr * BS:(lr + 1) * BS, dhi],
                                    start=(kc == 0),
                                    stop=(kc == 3),
                                )
                        pre = gate.tile([128, 2 * BS], F32, name=f"pre{g}_{s}",
                                        tag=f"pre{g}")
                        nc.vector.tensor_add(
                            pre[:], psg[:], xpt[:, g * 2 * BS:(g + 1) * 2 * BS])
                        t = gate.tile([128, 2 * BS], F32, name=f"tg{g}_{s}",
                                      tag=f"tg{g}")
                        nc.scalar.activation(
                            t[:], pre[:], AF.Tanh if g == 0 else AF.Sigmoid)
                        tg[g] = t
                        # interleave the cn chain as its operands appear
                        if g == 1:
                            nc.vector.tensor_mul(cn[:], tg[1][:], tg[0][:])
                        elif g == 2:
                            nc.vector.tensor_mul(t2[:], tg[2][:], lc)
                            nc.vector.tensor_add(cn[:], cn[:], t2[:])
                        elif g == 3:
                            nc.vector.tensor_mul(t3[:], tg[3][:], rc)
                            nc.vector.tensor_add(cn[:], cn[:], t3[:])
                            # c writeback first: next step's c-gather needs it
                            nc.vector.tensor_tensor(
                                out=st_c[:, rows, :].rearrange("p i c -> p c i"),
                                in0=cn[:].rearrange("p (c b) -> p c b", c=2),
                                in1=mb, op=OP.mult,
                            )
                            nc.scalar.activation(tc_t[:], cn[:], AF.Tanh)
                    hn = gate.tile([128, 2 * BS], F32, tag="hn")
                    nc.vector.tensor_mul(hn[:], tg[4][:], tc_t[:])
                    nc.vector.tensor_tensor(
                        out=st_h[:, rows, :].rearrange("p i c -> p c i"),
                        in0=hn[:].rearrange("p (c b) -> p c b", c=2),
                        in1=mb, op=OP.mult,
                    )
                    hn_m = gate.tile([128, 2 * BS], BF16, tag="hn_m")
                    nc.vector.tensor_tensor(
                        out=hn_m[:].rearrange("p (c b) -> p c b", c=2),
                        in0=hn[:].rearrange("p (c b) -> p c b", c=2),
                        in1=mb, op=OP.mult,
                    )
                    nc.sync.dma_start(
                        out=d_out[s].rearrange("c p b -> p c b"),
                        in_=hn_m[:].rearrange("p (c b) -> p c b", c=2),
                    )
            _pstack.close()
            xbA_cm.__exit__(None, None, None)
            stA_cm.__exit__(None, None, None)
            psA_cm.__exit__(None, None, None)
            phA_cm.__exit__(None, None, None)
    nc.compile()
    return nc


def _prep_core_inputs(x, x_mask, li, ri, Wx, bx, Wlh, Wrh, core):
    b0 = core * BS
    bf16 = np.float16

    xr = x[:, b0:b0 + BS, :]                       # [S, BS, D]
    xT = np.ascontiguousarray(xr.transpose(2, 0, 1))  # [D, S, BS]
    xT = xT.reshape(2, 128, NE).transpose(1, 0, 2)    # [128, 2, NE]; d = dhi*128+p
    xT = np.ascontiguousarray(xT).astype(np.float32)
    xTh = xT.astype(np.float16)
    xTl = (xT - xTh.astype(np.float32)).astype(np.float16)

    idx = np.zeros((128, S, 8), np.int16)
    lif = li[:, b0:b0 + BS] * BS + np.arange(BS)[None, :]
    rif = ri[:, b0:b0 + BS] * BS + np.arange(BS)[None, :]
    idxlist = np.concatenate([lif, rif], axis=1)   # [S, 128]
    for j in range(2 * BS):
        idx[np.arange(128) % 16 == (j % 16), :, j // 16] = idxlist[:, j][None, :]

    maskv = np.ascontiguousarray(
        x_mask[:, b0:b0 + BS].reshape(1, NE)).astype(np.float16)

    return {"xTh": xTh, "xTl": xTl, "idx": idx, "maskv": maskv}


def _prep_shared_inputs(Wx, bx, Wlh, Wrh):
    bf16 = np.float16
    # W2[zd, gk]: zd<256 -> Wlh[g,k,zd]; zd>=256 -> Wrh[g,k,zd-256]
    W2 = np.zeros((2 * D, GD), np.float32)
    for g in range(5):
        W2[:D, g * D:(g + 1) * D] = Wlh[g].T
        W2[D:, g * D:(g + 1) * D] = Wrh[g].T
    w2 = np.zeros((128, 4 * NMC, 128), np.float32)
    for mc in range(NMC):
        for kc in range(4):
            w2[:, mc * 4 + kc, :] = W2[kc * 128:(kc + 1) * 128,
                                       mc * 128:(mc + 1) * 128]
    WxM = np.zeros((D, XD), np.float32)
    for g in range(4):
        WxM[:, g * D:(g + 1) * D] = Wx[g].T
    wx = np.zeros((128, 2 * XMC, 128), np.float32)
    for mc in range(XMC):
        for kc in range(2):
            wx[:, mc * 2 + kc, :] = WxM[kc * 128:(kc + 1) * 128,
                                        mc * 128:(mc + 1) * 128]
    bxf = bx.reshape(XD)                     # [4*256]
    bx8 = np.zeros((128, XMC), np.float32)
    for mc in range(XMC):
        bx8[:, mc] = bxf[mc * 128:(mc + 1) * 128]
    wxh = wx.astype(np.float16)
    wxl = (wx - wxh.astype(np.float32)).astype(np.float16)
    return {"w2": w2.astype(bf16), "wxh": wxh, "wxl": wxl, "bx8": bx8}


def kernel(x, x_mask, x_left_mask, x_right_mask, Wx, bx, Wlh, Wrh):
    x = np.asarray(x, np.float32)
    x_mask = np.asarray(x_mask, np.float32)
    li = np.argmax(np.asarray(x_left_mask), axis=-1).astype(np.int64)   # [S, B]
    ri = np.argmax(np.asarray(x_right_mask), axis=-1).astype(np.int64)
    Wx = np.asarray(Wx, np.float32)
    bx = np.asarray(bx, np.float32)
    Wlh = np.asarray(Wlh, np.float32)
    Wrh = np.asarray(Wrh, np.float32)

    if "nc" not in _CACHED:
        _CACHED["nc"] = build_program()
    nc = _CACHED["nc"]

    shared = _prep_shared_inputs(Wx, bx, Wlh, Wrh)
    in_maps = []
    for core in range(NCORES):
        m = _prep_core_inputs(x, x_mask, li, ri, Wx, bx, Wlh, Wrh, core)
        m.update(shared)
        in_maps.append(m)

    res = bass_utils.run_bass_kernel_spmd(nc, in_maps, core_ids=list(range(NCORES)))
    _CACHED["last_results"] = res

    out = np.empty((B, S, D), np.float32)
    for core in range(NCORES):
        hT = np.asarray(res.results[core]["hT"]).astype(np.float32)  # [S, 2, 128, BS]
        out[core * BS:(core + 1) * BS] = (
            hT.transpose(3, 0, 1, 2).reshape(BS, S, D))
    return out
